# revision 40
# baseline (speedup 1.0000x reference)
"""Trainium2 Bass kernel for nn_Attention3D: RMSNorm3D + 8-head attention + out-proj.

Sharding: 16 (b, h) slices over 8 cores -> each core gets one batch b and two
heads (h0, h0+1). Per-core weights are sliced/folded on the host.

Device pipeline per core:
  - norm: ||x_n|| via PE ones-reductions, xn = x * inv_bcast
  - qkv projections in fp32r; q/k converted to fp8e4 (scores in DoubleRow fp8
    with a zero plane + a constant bias-plane row so the softmax shift rides
    the matmul); v converted to fp8e4 with a ones column (denominator row)
  - scores s' = q8.k8 + b0 accumulated f32 in PSUM
  - softmax exp split across two engines:
      ACT tiles: p = Exp(s' + bias) -> fp8e4 (true exp, e4m3 rounding)
      DVE tiles: p = bits_u8(round(max(s',0) * 8*log2e)) == Schraudolph exp
        directly in e4m3 bit space (negative scores clamp to +0)
  - attn@V in fp8 DoubleRow (2 key-tiles contracted per pass, 0.5 cyc/row):
    stationary [128, 2, 128] = [v_h | ones | zeros]; row 64 of o accumulates
    the softmax denominator Z
  - out-proj per head in fp32r on the UNNORMALIZED o; host divides by Z
    (the division is per-(head, query) so it cannot ride the fused per-core
    projection; dividing on-device costs 3 DVE ops/stage vs 0 on host)
Host: y[b] = sum_h (y_partial[core, h] / Z[core, h]) + b_out.
"""
from contextlib import ExitStack

import numpy as np

import concourse.bass as bass
import concourse.tile as tile
from concourse import bacc, mybir
from concourse.bass_utils import run_bass_kernel_spmd

F32 = mybir.dt.float32
F32R = mybir.dt.float32r
F8 = mybir.dt.float8e4
U8 = mybir.dt.uint8
AF = mybir.ActivationFunctionType
Alu = mybir.AluOpType
DR = mybir.MatmulPerfMode.DoubleRow


B, C, H, W, D = 2, 256, 16, 16, 16
N = H * W * D            # 4096
HEADS, DH = 8, 64
HID = HEADS * DH         # 512
NCORES = 8

ICH = 1024               # query-chunk (free dim of scores psum)
NIC = N // ICH           # 4
NJ = N // 128            # 32 key tiles
NG = NJ // 2             # 16 DoubleRow key groups

# Schraudolph / fp8 constants
A_SCH = float(8 * np.log2(np.e))        # 11.5416
SH = 2.0                                 # exp shift: p = exp(s - SH) fits e4m3
B_DVE = 56.0 - A_SCH * SH                # u8-Schraudolph bias (neg -> sat 0)
ACT_DELTA = 0.06                         # ACT-vs-rounding-Schraudolph centering
BIAS_ACT = -SH + ACT_DELTA
E4M3_ONE = 0x38                          # 1.0
ONE_F32_BITS = 0x3F800000
U32 = mybir.dt.uint32

# per-stage exp-tile engine assignment (by stage parity to balance ACT/DVE)
ACT_SET_EVEN = frozenset(set(range(0, NJ, 2)) | {1, 17})
ACT_SET_ODD = frozenset(set(range(0, NJ, 2)) | {1})



def _copy(eng, nc, out, in_):
    """Engine-dispatching copy (ACT has no tensor_copy)."""
    if eng is nc.scalar:
        nc.scalar.copy(out, in_)
    else:
        eng.tensor_copy(out, in_)

def build_attention_body(nc, tc, ctx, xb, wa, wv, wo, y, z, dbg=None):
    const = ctx.enter_context(tc.tile_pool(name="const", bufs=1))
    work = ctx.enter_context(tc.tile_pool(name="work", bufs=2))
    pwork = ctx.enter_context(tc.tile_pool(name="pwork", bufs=3))
    psum = ctx.enter_context(tc.tile_pool(name="psum", bufs=2, space="PSUM"))

    # ---- constants / big slabs ----
    x_sb = const.tile([128, 2, N], F32R, tag="x")           # x, c-tile major
    wa_sb = const.tile([128, 2, 256], F32R, tag="wa")       # [q|k] proj [c,(ct,o)]
    wv_sb = const.tile([128, 2, 256], F32R, tag="wv")       # W_V^T zero-padded
    wo_sb = const.tile([64, 2, 256], F32R, tag="wo")        # W_O^T [d, h, c]
    ones_col = const.tile([128, 1], F32R, tag="onesc")
    ones_row = const.tile([1, 128], F32R, tag="onesr")
    invn_row = const.tile([1, N], F32R, tag="invr")         # 1/||x_n|| row
    inv_bcast = const.tile([128, N], F32, tag="invb")       # bcast over parts
    nrm_col = const.tile([128, NJ], F32, tag="nrmc")        # ||x|| per key, tiled
    invn_col = const.tile([128, NJ], F32, tag="invc")       # 1/||x|| per key
    ainv_col = const.tile([128, NJ], F32, tag="ainvc")      # A_SCH/||x|| per key
    bias_act = const.tile([128, 1], F32, tag="bact")        # ACT exp bias
    qk_slab = const.tile([128, 2, N], F32R, tag="qk")       # [:,0]=q^T [:,1]=k^T
    v8 = const.tile([128, NG, 2, 2, 128], F8, tag="v8")     # [k, g, pl, h, col]
    zpad = const.tile([128, 2, 128], F8, tag="zpad")        # zero stationary

    nc.vector.memset(ones_col[:].bitcast(U32), ONE_F32_BITS)
    nc.vector.memset(ones_row[:].bitcast(U32), ONE_F32_BITS)
    nc.vector.memset(bias_act[:], BIAS_ACT)
    # v8: zeros (cols 65-127 pad the DoubleRow stationary to the legal 128),
    # ones denominator column. Pool engine - no input deps, runs at t=0.
    nc.gpsimd.memset(zpad[:].bitcast(U8), 0)
    # v8 pad cols 65..127 stay uninitialized: they only feed o_ps partitions
    # 65..127, which are never read back
    nc.gpsimd.memset(v8[:, :, :, :, 64:65].bitcast(U8), E4M3_ONE)

    # ---- input DMAs: x spread over 4 queues so all chunks land early ----
    _dma_eng = [nc.sync, nc.gpsimd, nc.scalar, nc.sync]
    for ch in range(8):
        for ct in range(2):
            _dma_eng[(2 * ch + ct) % 4].dma_start(
                out=x_sb[:, ct, ch * 512:(ch + 1) * 512],
                in_=xb[ct * 128:(ct + 1) * 128, ch * 512:(ch + 1) * 512],
            )
    for ct in range(2):
        nc.sync.dma_start(out=wa_sb[:, ct, :], in_=wa[ct * 128:(ct + 1) * 128, :])
        nc.gpsimd.dma_start(out=wv_sb[:, ct, :], in_=wv[ct * 128:(ct + 1) * 128, :])
    nc.sync.dma_start(out=wo_sb[:, :, :], in_=wo.rearrange("(d h) c -> d h c", h=2))

    def norm_bundle(ch, ptag, eng, width=512):
        """Norms for one `width`-col span at chunk ch: row form (for the q
        broadcast) and per-key column form (consumed as per-partition scales
        by exp/v8 - the key-side normalization rides those ops for free)."""
        sl = slice(ch * 512, ch * 512 + width)
        nw = width // 512
        nr_ps = psum.tile([1, width], F32, tag=ptag, bufs=3, name=f"nr_ps_{ch}")
        nc_ps = psum.tile([128, 4 * nw], F32, tag=ptag, bufs=3,
                          name=f"nc_ps_{ch}")
        for w in range(nw):
            wsl = bass.ts(ch + w, 512)
            x2c = [work.tile([128, 512], F32R, tag="x2", bufs=4,
                             name=f"x2_{ch + w}_{i}") for i in range(2)]
            nc.gpsimd.tensor_mul(x2c[0][:], x_sb[:, 0, wsl], x_sb[:, 0, wsl])
            nc.vector.tensor_mul(x2c[1][:], x_sb[:, 1, wsl], x_sb[:, 1, wsl])
            for ct in range(2):
                nc.tensor.matmul(nr_ps[0:1, w * 512:(w + 1) * 512], ones_col[:],
                                 x2c[ct][:], start=(ct == 0), stop=(ct == 1))
            for tt in range(4):
                for ct in range(2):
                    nc.tensor.matmul(nc_ps[:, 4 * w + tt:4 * w + tt + 1],
                                     x2c[ct][:, tt * 128:(tt + 1) * 128]
                                     .bitcast(F32),
                                     ones_col[:].bitcast(F32),
                                     start=(ct == 0), stop=(ct == 1))
        nrm_c = work.tile([1, width], F32, tag="nr", name=f"nrm_c_{ch}")
        nc.scalar.activation(out=nrm_c[:], in_=nr_ps[:], func=AF.Sqrt)
        nc.vector.reciprocal(out=invn_row[0:1, sl], in_=nrm_c[:])
        csl = slice(ch * 4, ch * 4 + 4 * nw)
        nc.scalar.activation(out=nrm_col[:, csl], in_=nc_ps[:], func=AF.Sqrt)
        nc.vector.reciprocal(out=invn_col[:, csl], in_=nrm_col[:, csl])
        nc.vector.tensor_scalar_mul(out=ainv_col[:, csl], in0=invn_col[:, csl],
                                    scalar1=A_SCH)
        for w in range(nw):
            wsl = bass.ts(ch + w, 512)
            ib_ps = psum.tile([128, 512], F32, tag=ptag, bufs=3,
                              name=f"ib_ps_{ch + w}")
            nc.tensor.matmul(ib_ps[:], ones_row[:], invn_row[0:1, wsl])
            nc.vector.tensor_copy(inv_bcast[:, wsl], ib_ps[:])

    def qk_bundle(ch, which, ptag, eng):
        """q or k projection for one 512-col chunk (PSUM -> f32r slab).
        q (which=0) is normalized per query via inv_bcast; k stays raw (its
        per-key 1/||x|| rides the exp scale operand)."""
        def emit():
            sl = bass.ts(ch, 512)
            osl = slice(which * 128, which * 128 + 128)
            qk_ps = psum.tile([128, 512], F32, tag=ptag, bufs=3,
                              name=f"qk_ps_{ch}_{which}")
            for ct in range(2):
                nc.tensor.matmul(qk_ps[:], wa_sb[:, ct, osl],
                                 x_sb[:, ct, sl], start=(ct == 0), stop=(ct == 1))
            if which == 0:
                nc.vector.tensor_mul(qk_slab[:, 0, sl], qk_ps[:],
                                     inv_bcast[:, sl])
            else:
                _copy(eng, nc, qk_slab[:, 1, sl], qk_ps[:])
        return emit

    def v_bundle(t, ptag, eng):
        """V projection from raw x + per-key normalize + fp8 convert."""
        def emit():
            v_ps = psum.tile([128, 256], F32, tag=ptag, bufs=3, name=f"v_ps_{t}")
            for ct in range(2):
                nc.tensor.matmul(v_ps[:], x_sb[:, ct, t * 128:(t + 1) * 128],
                                 wv_sb[:, ct, :], start=(ct == 0), stop=(ct == 1))
            if eng is nc.scalar:
                nc.scalar.activation(
                    out=v8[:, t // 2, t % 2, :, 0:64].bitcast(F8),
                    in_=v_ps[:, 0:128].rearrange("p (h d) -> p h d", h=2),
                    func=AF.Copy, scale=invn_col[:, t:t + 1])
            else:
                eng.tensor_scalar_mul(
                    out=v8[:, t // 2, t % 2, :, 0:64].bitcast(F8),
                    in0=v_ps[:, 0:128].rearrange("p (h d) -> p h d", h=2),
                    scalar1=invn_col[:, t:t + 1])
        return emit

    def outproj_piece(ic, o_slab, hh, mt):
        def emit():
            y_ps = psum.tile([128, ICH], F32, tag="s", bufs=3,
                             name=f"y_ps_{ic}_{hh}_{mt}")
            for cc in range(2):
                nc.tensor.matmul(
                    y_ps[:, cc * 512:(cc + 1) * 512],
                    wo_sb[:, hh, mt * 128:(mt + 1) * 128],
                    o_slab[0:64, hh, cc * 512:(cc + 1) * 512])
            y_ev = pwork.tile([128, ICH], F32, tag="yev", bufs=3,
                              name=f"y_ev_{ic}_{hh}_{mt}")
            eng = nc.vector if (mt + hh) % 2 else nc.scalar
            _copy(eng, nc, y_ev[:], y_ps[:])
            nc.sync.dma_start(
                out=y[hh, mt * 128:(mt + 1) * 128, ic * ICH:(ic + 1) * ICH],
                in_=y_ev[:])
        return emit

    # prefix: everything before the attention stages, ordered chunk-by-chunk
    # so each engine queue follows data readiness (no deferral into the
    # stages - the in-order engine queues head-of-line block on cross-engine
    # chains when setup work is interleaved with the exp stream)
    # full prefix: all norms (the only Sqrt users - keep them ahead of the
    # exp stream so the ACT function table loads exactly once), projections
    # and fp8 staging, pipelined chunk-by-chunk on the fast engines
    _cv = [nc.vector, nc.scalar]
    norm_bundle(0, "s", 0, width=1024)
    for ch in (0, 1):
        qk_bundle(ch, 1, "s", _cv[ch % 2])()
        qk_bundle(ch, 0, "s", _cv[(ch + 1) % 2])()
        for t in range(4 * ch, 4 * ch + 4):
            v_bundle(t, "s", _cv[t % 2])()
    norm_bundle(2, "s", 0, width=1024)
    norm_bundle(4, "s", 0, width=1024)
    norm_bundle(6, "s", 0, width=1024)

    # ch2-7 projections/staging interleave into the early stages, landing
    # just before their first consumers
    deferred = {}

    def _defer(key, *fns):
        deferred.setdefault(key, []).extend(fns)

    for ch in range(2, 8):
        _defer((0, 4 * ch - 6), qk_bundle(ch, 1, "s", _cv[ch % 2]))
    for t in range(8, 32):
        _defer((0, min(t + 4, 29)), v_bundle(t, "s", _cv[t % 2]))
    for ch in range(2, 8):
        si = 2 * (ch // 2) - 1
        _defer((si, 6 + 6 * (ch % 2)), qk_bundle(ch, 0, "s", _cv[(ch + 1) % 2]))

    stages = [(ic, h) for ic in range(NIC) for h in range(2)]
    o_slab = None
    for si, (ic, h) in enumerate(stages):
        if h == 0:
            o_slab = work.tile([65, 2, ICH], F32R, tag="osl", name=f"osl_{ic}")
        hsl = slice(h * 64, (h + 1) * 64)
        o_ps = psum.tile([128, ICH], F32, tag="o", bufs=1,
                         name=f"o_ps_{ic}_{h}")
        pq = {}
        for j in range(NJ):
            g, pl = j // 2, j % 2
            if pl == 0:
                pq[g] = pwork.tile([128, 2, ICH], U8, tag="p", bufs=5,
                                   name=f"p_{ic}_{h}_{g}")
            s_ps = psum.tile([128, ICH], F32, tag="s", bufs=3,
                             name=f"s_{ic}_{h}_{j}")
            for hf in range(2):
                nc.tensor.matmul(
                    s_ps[:, hf * 512:(hf + 1) * 512],
                    qk_slab[hsl, 1, j * 128:(j + 1) * 128],
                    qk_slab[hsl, 0, ic * ICH + hf * 512:ic * ICH + (hf + 1) * 512])
            for fn in deferred.pop((si, j), []):
                fn()
            if dbg is not None and si == 0 and j == 4:
                nc.sync.dma_start(out=dbg["p00"], in_=pq[0][:])
                nc.sync.dma_start(out=dbg["p01"], in_=pq[1][:])
            if j == 9:
                # the opening DoubleRow matmul of a PSUM accumulation group
                # mis-accumulates on hardware: open with a batch of
                # zero-stationary passes (all 4 regions, before any real
                # group), then every real group accumulates with start=False
                for qc in range(4):
                    nc.tensor.matmul(
                        o_ps[:, qc * 256:(qc + 1) * 256],
                        zpad[:],
                        pq[0][:, :, qc * 256:(qc + 1) * 256].bitcast(F8),
                        perf_mode=DR, start=True, stop=False)
            if j >= 9 and j % 2 == 1:
                ag = (j - 9) // 2
                for qc in range(4):
                    nc.tensor.matmul(
                        o_ps[:, qc * 256:(qc + 1) * 256],
                        v8[:, ag, :, h, :],
                        pq[ag][:, :, qc * 256:(qc + 1) * 256].bitcast(F8),
                        perf_mode=DR, start=False, stop=False)
            if dbg is not None and si == 0 and j == 0:
                s_dbg = work.tile([128, ICH], F32, tag="sdbg", name="s_dbg")
                nc.vector.tensor_copy(s_dbg[:], s_ps[:])
                nc.sync.dma_start(out=dbg["s00"], in_=s_dbg[:])
            act_set = ACT_SET_EVEN if si % 2 == 0 else ACT_SET_ODD
            if j in act_set:
                nc.scalar.activation(out=pq[g][:, pl, :].bitcast(F8),
                                     in_=s_ps[:], func=AF.Exp, bias=bias_act[:],
                                     scale=invn_col[:, j:j + 1])
            else:
                nc.vector.tensor_scalar(out=pq[g][:, pl, :], in0=s_ps[:],
                                        scalar1=ainv_col[:, j:j + 1],
                                        scalar2=B_DVE,
                                        op0=Alu.mult, op1=Alu.add)

        def tail_avs(h, o_ps, pq):
            def emit():
                for ag in range(NG - 4, NG):
                    for qc in range(4):
                        nc.tensor.matmul(
                            o_ps[:, qc * 256:(qc + 1) * 256],
                            v8[:, ag, :, h, :],
                            pq[ag][:, :, qc * 256:(qc + 1) * 256].bitcast(F8),
                            perf_mode=DR, start=False, stop=(ag == NG - 1))
            return emit

        def o_copy(h, ic, o_ps, o_slab):
            def emit():
                eng = nc.vector if h else nc.scalar
                _copy(eng, nc, o_slab[:, h, :], o_ps[0:65, :])
                nc.sync.dma_start(out=z[h, ic * ICH:(ic + 1) * ICH],
                                  in_=o_slab[64:65, h, :].bitcast(F32))
            return emit

        deferred.setdefault((si + 1, 1), []).append(tail_avs(h, o_ps, pq))
        deferred.setdefault((si + 1, 3), []).append(o_copy(h, ic, o_ps, o_slab))
        if h == 1:
            for pi, (hh, mt) in enumerate([(0, 0), (0, 1), (1, 0), (1, 1)]):
                deferred.setdefault((si + 1, 13 + 4 * pi), []).append(
                    outproj_piece(ic, o_slab, hh, mt))
            if dbg is not None and ic == 0:
                def dump_o(o_slab):
                    def emit():
                        nc.sync.dma_start(out=dbg["o0"], in_=o_slab[:].bitcast(F32))
                    return emit
                deferred.setdefault((si + 1, 9), []).append(dump_o(o_slab))
    for key in sorted(deferred):
        for fn in deferred[key]:
            fn()
    if dbg is not None:
        nc.sync.dma_start(out=dbg["qk"], in_=qk_slab[:].bitcast(F32))
        nc.sync.dma_start(out=dbg["v8"], in_=v8[:].bitcast(U8))
        nc.sync.dma_start(out=dbg["xn"], in_=xn_sb[:].bitcast(F32))


_NC_CACHE = None


def _build():
    global _NC_CACHE
    if _NC_CACHE is not None:
        return _NC_CACHE
    nc = bacc.Bacc("TRN2", target_bir_lowering=False, debug=False, num_devices=NCORES)
    xb = nc.dram_tensor("xb", [C, N], F32R, kind="ExternalInput").ap()
    wa = nc.dram_tensor("wa", [C, 256], F32R, kind="ExternalInput").ap()
    wv = nc.dram_tensor("wv", [C, 256], F32R, kind="ExternalInput").ap()
    wo = nc.dram_tensor("wo", [128, C], F32R, kind="ExternalInput").ap()
    y = nc.dram_tensor("y", [2, C, N], F32, kind="ExternalOutput").ap()
    z = nc.dram_tensor("z", [2, N], F32, kind="ExternalOutput").ap()
    import os
    dbg = None
    if os.environ.get("K_DEBUG"):
        dbg = {
            "s00": nc.dram_tensor("s00", [128, ICH], F32, kind="ExternalOutput").ap(),
            "p00": nc.dram_tensor("p00", [128, 2, ICH], U8, kind="ExternalOutput").ap(),
            "p01": nc.dram_tensor("p01", [128, 2, ICH], U8, kind="ExternalOutput").ap(),
            "qk": nc.dram_tensor("qk", [128, 2, N], F32, kind="ExternalOutput").ap(),
            "v8", "v8d": None,
        }
    with tile.TileContext(nc) as tc, ExitStack() as ctx:
        with nc.allow_low_precision(reason="fp8 attention within tolerance"):
            build_attention_body(nc, tc, ctx, xb, wa, wv, wo, y, z)
    nc.compile()
    _NC_CACHE = nc
    return nc


def _host_prep(x, g, w_qkv, w_out):
    """Per-core input maps."""
    x = np.ascontiguousarray(np.asarray(x, np.float32))
    g = np.asarray(g, np.float32)
    w_qkv = np.asarray(w_qkv, np.float32)
    w_out = np.asarray(w_out, np.float32)

    Wg = w_qkv * (g * np.sqrt(np.float32(C)))[None, :]
    Wq = Wg[0:HID] * np.float32(DH ** -0.5)
    Wk = Wg[HID:2 * HID]
    Wv = Wg[2 * HID:3 * HID]

    in_maps = []
    for core in range(NCORES):
        b = core // 4
        h0 = 2 * (core % 4)
        sl = slice(h0 * DH, (h0 + 2) * DH)
        W_A = np.concatenate([Wq[sl], Wk[sl]], 0)            # [256, 256]
        wo_slice = w_out[:, sl]                              # [256, 128]
        wo_dev = np.ascontiguousarray(
            wo_slice.T.reshape(2, DH, C).transpose(1, 0, 2).reshape(128, C))
        in_maps.append({
            "xb": np.ascontiguousarray(x[b].reshape(C, N)),
            "wa": np.ascontiguousarray(W_A.T),               # [c, o]
            "wv": np.ascontiguousarray(np.pad(Wv[sl].T, ((0, 0), (0, 128)))),
            "wo": wo_dev,                                    # [(d,h), c]
        })
    return in_maps


_RUNNER_CACHE = None


def _make_runner(nc):
    """Build the sharded PJRT callable once; reuse across kernel() calls
    (run_bass_kernel_spmd re-traces jax on every invocation)."""
    import jax
    from jax.sharding import Mesh, PartitionSpec
    from jax.experimental.shard_map import shard_map
    from concourse import bass2jax

    bass2jax.install_neuronx_cc_hook()
    in_names, out_names, out_avals, zero_outs = [], [], [], []
    for alloc in nc.m.functions[0].allocations:
        if not isinstance(alloc, mybir.MemoryLocationSet):
            continue
        name = alloc.memorylocations[0].name
        if alloc.kind == "ExternalInput":
            if nc.partition_id_tensor is None or name != nc.partition_id_tensor.name:
                in_names.append(name)
        elif alloc.kind == "ExternalOutput":
            out_names.append(name)
            shape = tuple(alloc.tensor_shape)
            dtype = mybir.dt.np(alloc.dtype)
            out_avals.append(jax.core.ShapedArray(shape, dtype))
            zero_outs.append(np.zeros(shape, dtype))
    n_params = len(in_names)
    all_in_names = list(in_names) + list(out_names)
    if nc.partition_id_tensor is not None:
        all_in_names.append(nc.partition_id_tensor.name)

    def _body(*args):
        operands = list(args)
        if nc.partition_id_tensor is not None:
            operands.append(bass2jax.partition_id_tensor())
        return tuple(bass2jax._bass_exec_p.bind(
            *operands,
            out_avals=tuple(out_avals),
            in_names=tuple(all_in_names),
            out_names=tuple(out_names),
            lowering_input_output_aliases=(),
            sim_require_finite=True,
            sim_require_nnan=True,
            nc=nc,
        ))

    devices = jax.devices()[:NCORES]
    mesh = Mesh(np.asarray(devices), ("core",))
    n_outs = len(out_avals)
    fn = jax.jit(
        shard_map(_body, mesh=mesh,
                  in_specs=(PartitionSpec("core"),) * (n_params + n_outs),
                  out_specs=(PartitionSpec("core"),) * n_outs,
                  check_rep=False),
        keep_unused=True,
    )
    sharding = jax.sharding.NamedSharding(mesh, PartitionSpec("core"))
    dev_zero = [jax.device_put(
        np.zeros((NCORES * z.shape[0], *z.shape[1:]), z.dtype), sharding)
        for z in zero_outs]

    def run(in_maps):
        concat_in = [np.concatenate([np.asarray(m[name]) for m in in_maps], axis=0)
                     for name in in_names]
        dev_in = [jax.device_put(a, sharding) for a in concat_in]
        outs = fn(*dev_in, *dev_zero)
        names = list(out_names)
        res = {}
        for nm, o in zip(names, outs):
            res[nm] = np.asarray(o)
        yc = res["y"].reshape(NCORES, 2, C, N)
        zc = res["z"].reshape(NCORES, 2, N)
        return yc, zc

    return run


def kernel(x, g, w_qkv, w_out, b_out):
    global _RUNNER_CACHE
    nc = _build()
    in_maps = _host_prep(x, g, w_qkv, w_out)
    try:
        if _RUNNER_CACHE is None:
            _RUNNER_CACHE = _make_runner(nc)
        yc, zc = _RUNNER_CACHE(in_maps)
    except Exception:
        res = run_bass_kernel_spmd(nc, in_maps, core_ids=list(range(NCORES)))
        yc = np.stack([res.results[c]["y"] for c in range(NCORES)])
        zc = np.stack([res.results[c]["z"] for c in range(NCORES)])
    y = np.zeros((B, C, N), np.float32)
    for core in range(NCORES):
        b = core // 4
        y[b] += (yc[core] / zc[core][:, None, :]).sum(axis=0)
    y += np.asarray(b_out, np.float32)[None, :, None]
    return y.reshape(B, C, H, W, D)
